# revision 55
# baseline (speedup 1.0000x reference)
"""Multi-head attention (b=2, n=2048, dim=1024, h=16) on 8 TRN2 NeuronCores.

Sharding: tensor-parallel over heads x data-parallel over batch.
Core c handles batch c//4 and head-group c%4 (4 heads of 64 dims each).
Each core computes its QKV projection slice, local attention for its 4
heads, and a partial output projection (row-slice of W_out); the host
sums the 4 partials per batch and adds b_out.

Per-core kernel layout (all matmul operands bf16, fp32 PSUM accumulate):
  - x^T is staged in SBUF as [128, kt*2048+i] so it serves both as the
    moving operand of the q/k projections (q^T/k^T in [d, n] layout) and
    as the stationary operand of the v projection (V in [n, d] layout).
  - scores are computed transposed (S^T[j, i] = k_j . q_i) so softmax's
    denominator comes free from the PV matmul: V is augmented with a
    ones column, so O_aug = [V|1]^T @ P gives O^T rows 0..63 and the
    softmax denominator in row 64.
  - score psum tiles are [128, 512] HALVES on a 6-deep one-bank ring
    (not 3x[128,1024]): each slot frees after a ~0.6us half-exp instead
    of a ~1.2us full-tile exp, which halves the exp latency the in-order
    ST stream is coupled to through psum WAR, and breaks the
    3-allocations-per-iteration resonance that parked every projection
    fill on the same slot.
  - exp runs on ScalarE out of PSUM, with a per-(stage, jt) subset
    offloaded to a one-instruction Schraudolph bit-exp on DVE to keep
    both engines under the PE pace.
  - normalization multiplies O^T by 1/denom: heads 0/1 entirely on
    gpsimd (partition_broadcast + tensor_tensor, engine otherwise idle),
    heads 2/3 via a ones-matmul partition-broadcast + DVE multiply since
    they feed W_out almost immediately and gpsimd ops land several us
    after their dependencies.
  - each stage's last 8 PVs run c2-separated at jt2=10 so the finish
    chain (osb cast on DVE/Act + reciprocal) overlaps ST(12..15) and the
    next stage's PV run never WAR-stalls on the 2 rotating oa banks.
  - W_out tiles 0..7 interleave with the last stage's PV drain (their OT
    inputs are final at finish(6)), starting the 4MB output-DMA stream
    ~5us early; the last two tiles DMA per 512-column half.
"""

import sys

sys.path.insert(0, "/opt/trn_rl_repo")

import numpy as np
import ml_dtypes

B, N, DIM, H = 2, 2048, 1024, 16
D = DIM // H            # 64 head dim
NCORES = 8
HPC = 4                 # heads per core
DL = HPC * D            # 256 local head dims per core
KT = DIM // 128         # 8 contraction tiles for projections
NT = N // 128           # 16 n tiles
QW = KT * 512           # columns per i-quarter in the packed x^T layout
SCALE = D ** -0.5       # 0.125, folded into Wq host-side (exact power of 2)

_cached_nc = None


def _build_nc():
    # NOTE: tried flipping walrus --enable-ldw-opt=true (would let LDWEIGHTS
    # double-buffer across back-to-back matmuls): codegen rejects it
    # (visitInstLdweights error), so LDWEIGHTS stays per-matmul.
    from concourse import bacc, mybir, tile

    bf16 = mybir.dt.bfloat16
    f32 = mybir.dt.float32
    Exp = mybir.ActivationFunctionType.Exp
    mult = mybir.AluOpType.mult
    alu_add = mybir.AluOpType.add

    # Schraudolph bit-trick exp, one DVE instruction: the bf16 BIT PATTERN of
    # exp(x) is int((x*2^23/ln2 + (127*2^23 - 486411)) / 2^16), so a single
    # tensor_scalar (mult, add) with uint16 output dtype computes exp directly
    # (~1.8% rms / 4.2% max rel err); PV then reads the tile bitcast as bf16.
    # Offloads part of the exp stream from the Activation engine (the
    # attention-phase bottleneck: exp of a [128,1024] tile costs ~1.28us on
    # Act vs ~0.87us of PE work per tile at full clock) to DVE slack.
    EXP_A16 = float((1 << 23) / np.log(2.0) / 65536.0)
    EXP_B16 = float((127.0 * (1 << 23) - 486411.0) / 65536.0)
    u16 = mybir.dt.uint16
    # (stage, jt) set for bit-exp offload; Act handles the rest. jt=14 is
    # offloaded in every stage so the last score-psum slots drain fast at
    # stage boundaries (the next stage's first STs WAR-wait on them).
    # Stages 0-2 run 15-16 exps against more PE work (projection fills), so
    # a few extra offloads keep Act from throttling the st ring; slots are
    # chosen to land in iterations whose DVE queue is light (away from the
    # dense v-cast iterations).
    bitexp = set()
    for _s in range(3, 8):
        bitexp.update((_s, _jt) for _jt in (2, 4, 7, 9, 13, 14))
    bitexp.update((0, _jt) for _jt in (3, 7, 9, 14))
    bitexp.update((1, _jt) for _jt in (3, 7, 14))
    bitexp.update((2, _jt) for _jt in (5, 9, 14))

    nc = bacc.Bacc(None, target_bir_lowering=False, debug=False)

    xt = nc.dram_tensor("xt", [128, KT * N], bf16, kind="ExternalInput")
    wq = nc.dram_tensor("wq", [128, KT * DL], bf16, kind="ExternalInput")
    wk = nc.dram_tensor("wk", [128, KT * DL], bf16, kind="ExternalInput")
    wv = nc.dram_tensor("wv", [128, KT * DL], bf16, kind="ExternalInput")
    wo = nc.dram_tensor("wo", [128, 2 * DIM], bf16, kind="ExternalInput")
    onesmask = nc.dram_tensor("onesmask", [128, HPC * 128], bf16, kind="ExternalInput")
    out = nc.dram_tensor("out", [N, DIM], bf16, kind="ExternalOutput")

    with tile.TileContext(nc) as tc:
        with (
            tc.tile_pool(name="wpool", bufs=1) as wpool,
            tc.tile_pool(name="qkvpool", bufs=1) as qkvpool,
            tc.tile_pool(name="ppool", bufs=22) as ppool,
            tc.tile_pool(name="opool", bufs=1) as opool,
            tc.tile_pool(name="outpool", bufs=6) as outpool,
        ):
            # ---- input DMAs (weights first so projections start ASAP) ----
            wq_sb = wpool.tile([128, KT * DL], bf16, tag="wq_sb")
            wk_sb = wpool.tile([128, KT * DL], bf16, tag="wk_sb")
            wv_sb = wpool.tile([128, KT * DL], bf16, tag="wv_sb")
            wo_sb = wpool.tile([128, 2 * DIM], bf16, tag="wo_sb")
            xt_sb = wpool.tile([128, KT * N], bf16, tag="xt_sb")
            # warm_sb memset is gpsimd's FIRST instruction: the gpsimd
            # sequencer boots ~1us before vector's, so the warmup matmul
            # chain (and the HAM clock ramp) starts that much earlier.
            # (Tried issuing the critical input DMAs from gpsimd instead of
            # sync: +9us — engine-triggered DMA takes a much slower path
            # than the statically-generated sync-engine descriptors.)
            warm_sb = wpool.tile([128, 512], bf16, tag="warm_sb")
            nc.gpsimd.memset(warm_sb[:], 0.0)
            # wk first: the prelude's first projection group is wk x q0, so
            # the critical DMA chain is wk (0.5MB) + quarter 0 (1MB); wq is
            # only needed one group (~1.7us) later.
            nc.sync.dma_start(out=wk_sb[:], in_=wk[:])
            # x^T arrives i-quarter-major: the prelude projections only read
            # quarters 0/1, so the first score tile unlocks after 2MB not 4MB.
            # All quarters land before wv: stage-0's kT c2/c3 fills consume q2
            # and q3 well before the v-projection fills (late stage 0) need wv.
            nc.sync.dma_start(out=xt_sb[:, 0:QW], in_=xt[:, 0:QW])
            nc.sync.dma_start(out=wq_sb[:], in_=wq[:])
            for q in (1, 2, 3):
                nc.sync.dma_start(
                    out=xt_sb[:, q * QW:(q + 1) * QW], in_=xt[:, q * QW:(q + 1) * QW]
                )
            nc.sync.dma_start(out=wv_sb[:], in_=wv[:])
            nc.sync.dma_start(out=wo_sb[:], in_=wo[:])

            # Every matmul in the kernel uses a (128, 128) PE tile config: the
            # first matmul after a tile-size change stretches 216->~335ns (the
            # next LDWEIGHTS can't pre-pipeline across a reconfig), so scores
            # use K=128 with zero-padded rows and the recip broadcast uses a
            # row-masked [128, 128] stationary instead of a [1, 64] one.
            #
            # q^T/k^T per head in [d, n] layout: [128, N] tiles, rows 0..63 =
            # head data, rows 64..127 = zeros (zero-padding the contraction).
            qT = [qkvpool.tile([128, N], bf16, tag=f"qT{h}", name=f"qT{h}") for h in range(HPC)]
            kT = [qkvpool.tile([128, N], bf16, tag=f"kT{h}", name=f"kT{h}") for h in range(HPC)]
            # only the zero-padding rows D:128 need the memset (rows 0:D are
            # always overwritten by the projection casts). Memset time is
            # column-dominated (~1.7us each regardless of engine), so split
            # across gpsimd and vector — both are idle until ~16us — instead
            # of an 8x1.75us serial gpsimd chain that ran into stage 0.
            for t in (kT[0], qT[0], kT[1], qT[1]):
                nc.gpsimd.memset(t[D:128, :], 0.0)
            for t in (kT[2], qT[2], kT[3], qT[3]):
                nc.vector.memset(t[D:128, :], 0.0)
            # V augmented with ones column: per n-tile jt, per head h the
            # columns [jt*260 + h*65, jt*260 + h*65 + 65) hold [V_h | 1].
            # memset on gpsimd (vaug isn't read until the stage-0 v fills at
            # ~30us; keeps vector free for the early projection casts).
            vaug = qkvpool.tile([128, NT * (DL + HPC)], bf16, tag="vaug")
            nc.gpsimd.memset(vaug[:], 1.0)

            # unnormalized O^T + denom, two heads stacked per tile; normalized
            # O^T head-pairs
            osb2 = [opool.tile([128, N], bf16, tag=f"osb{t}", name=f"osb{t}") for t in range(2)]
            OT = [opool.tile([128, N], bf16, tag=f"OT{i}", name=f"OT{i}") for i in range(2)]
            # recstack row 32h = head h's 1/denom (single-partition engine
            # writes must start at a 32-aligned partition); rest stays zero
            recstack = opool.tile([128, N], bf16, tag="recstack", name="recstack")
            nc.gpsimd.memset(recstack[:], 0.0)
            # heads 0/1 normalize on gpsimd (idle during the stages): 1/denom
            # lands in recb row 0, partition_broadcast fans it out, one
            # [64, N] tensor_tensor writes OT. Only the EARLY heads — the
            # gpsimd ops run with ~40us of slack; heads 2/3 keep the PE-bcast
            # + DVE path since they feed wout almost immediately.
            recb = opool.tile([1, N], bf16, tag="recb", name="recb")
            bct = opool.tile([128, N], bf16, tag="bct", name="bct")
            # bcast stationary: column block h is [128, 128] with ones in
            # row 32h cols 0..63, zero elsewhere -> out rows 0..63 =
            # recstack row 32h.
            # Host-built constant (single-partition memsets at partition>0 are
            # rejected by the BIR verifier); not needed until the first
            # normalize (~stage 2), so its DMA queues last.
            ones_mask = wpool.tile([128, HPC * 128], bf16, tag="ones_mask")
            nc.sync.dma_start(out=ones_mask[:], in_=onesmask[:])

            # One PSUM scope for projections + attention so they overlap.
            # The "st" tag (score-tile HALVES, projection groups, bcast, wout
            # all share its 6 rotating [128,512] one-bank slots); the two
            # [65,512] PV accumulators use the remaining 2 of 8 banks.
            # 6 half-slots instead of 3 full [128,1024] slots: each slot is
            # freed by a ~0.6us half-exp instead of a ~1.2us full exp, so the
            # ST stream is latency-coupled to half the exp time, and the
            # 3-allocations-per-fill-iteration resonance (fill pj always
            # landing on the same slot, WAR-stalling on the previous fill's
            # second cast) breaks.
            with (
                tc.tile_pool(name="stps", bufs=6, space="PSUM") as stps,
                tc.tile_pool(name="oaps", bufs=2, space="PSUM") as oaps,
            ):
                def proj_v():
                    for jt in range(NT):
                        pj = stps.tile([128, 512], f32, tag="st", name="pj")
                        for kt in range(KT):
                            nc.tensor.matmul(
                                pj[:, 0:DL],
                                xt_sb[:, (jt // 4) * QW + kt * 512 + (jt % 4) * 128:
                                       (jt // 4) * QW + kt * 512 + (jt % 4) * 128 + 128],
                                wv_sb[:, kt * DL:(kt + 1) * DL],
                                start=(kt == 0),
                                stop=(kt == KT - 1),
                            )
                        base = jt * (DL + HPC)
                        for h in range(HPC):
                            nc.vector.tensor_copy(
                                out=vaug[:, base + h * 65: base + h * 65 + D],
                                in_=pj[:, h * D:(h + 1) * D],
                            )

                def normalize(h, chunks=(0, 1, 2, 3)):
                    ht, ho = h // 2, (h % 2) * D
                    for c in chunks:
                        bc = stps.tile([128, 512], f32, tag="st", name="bc")
                        nc.tensor.matmul(
                            bc[:],
                            ones_mask[:, h * 128:(h + 1) * 128],
                            recstack[:, c * 512:(c + 1) * 512],
                            start=True,
                            stop=True,
                        )
                        nc.vector.tensor_tensor(
                            out=OT[ht][ho:ho + D, c * 512:(c + 1) * 512],
                            in0=osb2[h // 2][(h % 2) * D:(h % 2) * D + D,
                                            c * 512:(c + 1) * 512],
                            in1=bc[0:D, :],
                            op=mult,
                        )

                # attention stages s=(h, half); PV for stage s-1 is issued
                # interleaved with stage s's ST/exp so PE never waits on the
                # current stage's exp.
                stages = [(h, half) for h in range(HPC) for half in range(2)]
                p_tiles = {}
                oa_tiles = {}

                st_tiles = {}

                def issue_st_c2(s, jt, c2):
                    h, half = stages[s]
                    sth = stps.tile([128, 512], f32, tag="st", name="st")
                    i0 = half * 1024 + c2 * 512
                    nc.tensor.matmul(
                        sth[:],
                        kT[h][:, jt * 128:(jt + 1) * 128],
                        qT[h][:, i0:i0 + 512],
                        start=True,
                        stop=True,
                    )
                    st_tiles.setdefault((s, jt), [None, None])[c2] = sth

                def issue_st(s, jt):
                    issue_st_c2(s, jt, 0)
                    issue_st_c2(s, jt, 1)

                def issue_exp(s, jt):
                    halves = st_tiles.pop((s, jt))
                    if (s, jt) not in bitexp:
                        p_t = ppool.tile([128, 1024], bf16, tag="p", name="p")
                        for c2 in range(2):
                            nc.scalar.activation(
                                out=p_t[:, c2 * 512:(c2 + 1) * 512],
                                in_=halves[c2][:], func=Exp,
                            )
                        p_tiles[(s, jt)] = (p_t, False)
                    else:
                        p_t = ppool.tile([128, 1024], u16, tag="p16", name="p16", bufs=9)
                        for c2 in range(2):
                            nc.vector.tensor_scalar(
                                out=p_t[:, c2 * 512:(c2 + 1) * 512],
                                in0=halves[c2][:],
                                scalar1=EXP_A16, scalar2=EXP_B16,
                                op0=mult, op1=alu_add,
                            )
                        p_tiles[(s, jt)] = (p_t, True)

                def issue_pv_one(s, jt, c2):
                    h, half = stages[s]
                    if jt == 0 and c2 == 0:
                        oa_tiles[s] = [
                            oaps.tile([65, 512], f32, tag="oa", name="oa")
                            for _ in range(2)
                        ]
                    p_t, is_u16 = p_tiles[(s, jt)]
                    vbase = jt * (DL + HPC) + h * 65
                    src = p_t[:, c2 * 512:(c2 + 1) * 512]
                    if is_u16:
                        src = src.bitcast(bf16)
                    nc.tensor.matmul(
                        oa_tiles[s][c2][:],
                        vaug[:, vbase: vbase + 65],
                        src,
                        start=(jt == 0),
                        stop=(jt == NT - 1),
                    )
                    if c2 == 1:
                        p_tiles.pop((s, jt))

                def issue_pv(s, jt):
                    issue_pv_one(s, jt, 0)
                    issue_pv_one(s, jt, 1)

                def finish_stage(s):
                    # release the two oa psum banks fast: both [65,512] casts
                    # FIRST (c2=0 on DVE, c2=1 on Act, in parallel), then the
                    # full-tile recips (cheap: DVE per-op overhead dominates,
                    # 65 rows cost the same as 1) and the 1-row recstack
                    # copies. oa0 frees after CAST0+RECIP0 (~1.3us), oa1
                    # after CAST1 (Act) + RECIP1 (~2us) — vs the old 2.6us
                    # serial chain, so the next stage's PV run on the same 2
                    # rotating banks doesn't WAR-stall the PE.
                    h, half = stages[s]
                    oas = oa_tiles.pop(s)
                    recs = []
                    for c2, oa in enumerate(oas):
                        i0 = half * 1024 + c2 * 512
                        eng_cp = nc.vector.tensor_copy if c2 == 0 else nc.scalar.copy
                        eng_cp(
                            out=osb2[h // 2][(h % 2) * D:(h % 2) * D + D, i0:i0 + 512],
                            in_=oa[0:D, :],
                        )
                    for c2, oa in enumerate(oas):
                        i0 = half * 1024 + c2 * 512
                        # custom-DVE ops cannot shift partitions: compute
                        # 1/denom in place at partition 64, then cast-copy
                        # (plain copy can shift) to the bf16 row
                        rec = opool.tile([65, 512], f32, tag="rectmp", name="rec", bufs=3)
                        nc.vector.reciprocal_approx_fast(out=rec[:], in_=oa[:])
                        rdst = (recstack[h * 32:h * 32 + 1, i0:i0 + 512]
                                if h >= 2 else recb[0:1, i0:i0 + 512])
                        nc.vector.tensor_copy(out=rdst, in_=rec[D:D + 1, :])
                    if half == 1:
                        if h == HPC - 1:
                            normalize(h, (2, 3))
                        elif h == 2:
                            # head 2 stays on the PE-bcast + DVE path: tried
                            # gpsimd per-half too — its half-1 multiply
                            # completed ~1us before wout tile 8 consumes it,
                            # too thin a margin given gpsimd's multi-us
                            # post-dependency slip
                            for c, jslot in enumerate((1, 3, 7, 11)):
                                fill[(s + 2, jslot)] = (
                                    lambda h=h, c=c: normalize(h, (c,))
                                )
                        else:
                            nc.gpsimd.partition_broadcast(bct[:], recb[0:1, :])
                            nc.gpsimd.tensor_tensor(
                                out=OT[h // 2][(h % 2) * D:(h % 2) * D + D, :],
                                in0=osb2[h // 2][(h % 2) * D:(h % 2) * D + D, :],
                                in1=bct[(h % 2) * D:(h % 2) * D + D, :],
                                op=mult,
                            )
                    elif h == HPC - 1:
                        # head 3's first-half inputs are final a stage early;
                        # normalizing here shortens the tail's critical path
                        normalize(h, (0, 1))

                def proj_v_group(jt):
                    pj = stps.tile([128, 512], f32, tag="st", name="pj")
                    for kt in range(KT):
                        nc.tensor.matmul(
                            pj[:, 0:DL],
                            xt_sb[:, (jt // 4) * QW + kt * 512 + (jt % 4) * 128:
                                       (jt // 4) * QW + kt * 512 + (jt % 4) * 128 + 128],
                            wv_sb[:, kt * DL:(kt + 1) * DL],
                            start=(kt == 0),
                            stop=(kt == KT - 1),
                        )
                    base = jt * (DL + HPC)
                    for h in range(HPC):
                        nc.vector.tensor_copy(
                            out=vaug[:, base + h * 65: base + h * 65 + D],
                            in_=pj[:, h * D:(h + 1) * D],
                        )

                def proj_qk_group(w_sb, dest, mt, c, part=None):
                    # part=0 emits the first half of the contraction, part=1
                    # the second half + copy; None emits everything
                    if part != 1:
                        self_pj = stps.tile([128, 512], f32, tag="st", name="pj")
                        proj_qk_group.pj = self_pj
                    pj = proj_qk_group.pj
                    kts = {0: range(0, KT // 2), 1: range(KT // 2, KT), None: range(KT)}[part]
                    for kt in kts:
                        nc.tensor.matmul(
                            pj[:],
                            w_sb[:, kt * DL + mt * 128: kt * DL + mt * 128 + 128],
                            xt_sb[:, c * QW + kt * 512: c * QW + (kt + 1) * 512],
                            start=(kt == 0),
                            stop=(kt == KT - 1),
                        )
                    if part != 0:
                        # split the [128, 512] psum group into the two heads'
                        # zero-padded tiles (rows 64..127 stay zero)
                        for hh in range(2):
                            nc.vector.tensor_copy(
                                out=dest[2 * mt + hh][0:D, c * 512:(c + 1) * 512],
                                in_=pj[hh * D:(hh + 1) * D, :],
                            )

                # warm-up: dependency-free matmuls on memset scratch run
                # during the input-DMA wait so the PE clock-gate (HAM) is
                # already at full rate when the real projections start
                wm = stps.tile([128, 512], f32, tag="st", name="wm")
                NWARM = 23
                for i in range(NWARM):
                    nc.tensor.matmul(
                        wm[:],
                        warm_sb[:, 0:128],
                        warm_sb[:],
                        start=(i == 0),
                        stop=(i == NWARM - 1),
                    )

                # emission: minimal prelude (3 groups unlock stage-0's first
                # STs), every other projection group spread as just-in-time
                # PE filler across stages 0-2 so the exp stream starts early
                # and stays fed.
                # stage-0 fill order tracks the x^T quarter DMA arrivals:
                # quarter-0/1 groups (incl. the mt1 heads) run while q2/q3 are
                # still in flight; wk c2/c3 land just before ST(0, 8)/ST(0,12)
                # consume those kT columns.
                proj_qk_group(wk_sb, kT, 0, 0, None)
                proj_qk_group(wq_sb, qT, 0, 0, None)
                proj_qk_group(wq_sb, qT, 0, 1, None)
                # fill placement: measured JIT stalls show a consuming ST
                # starts ~60ns after its chunk's cast lands, so every fill
                # leads its consumer by >=2 iterations: kT chunk c fills at
                # slot 2(c-1) (consumed at jt2=4c), qT c2/c3 (stage 1's i
                # range) next, then five v groups (v0..v4) so stage 1's last
                # v fill (g15) is two iterations clear of the tail PVs that
                # now run at jt2=12.
                fill = {
                    (0, 0): lambda: proj_qk_group(wk_sb, kT, 0, 1, None),
                    (0, 2): lambda: proj_qk_group(wk_sb, kT, 0, 2, None),
                    (0, 4): lambda: proj_qk_group(wk_sb, kT, 0, 3, None),
                    (0, 6): lambda: proj_qk_group(wq_sb, qT, 0, 2, None),
                    (0, 8): lambda: proj_qk_group(wq_sb, qT, 0, 3, None),
                }
                for j in range(5):
                    fill[(0, 10 + j)] = (lambda j=j: proj_v_group(j))
                for j in range(11):
                    fill[(1, j)] = (lambda j=j: proj_v_group(j + 5))
                mt1 = [(wq_sb, qT, 1, c) for c in range(4)] + [(wk_sb, kT, 1, c) for c in range(4)]
                for i in range(8):
                    fill[(2, 2 * i)] = (lambda i=i: proj_qk_group(*mt1[i], None))
                # j-tiles are processed in PAIRS of score tiles with the
                # previous stage's PV batched into runs of 8 matmuls (4 tiles,
                # at jt2 = 4/8/12, lag 4). Same-shape runs matter: at every PE
                # shape alternation the next LDWEIGHTS fails to pre-pipeline
                # and the matmul stretches ~216->335ns, so cutting ST<->PV
                # boundaries from 2/tile to ~1/2-tiles buys back ~20us.
                # The previous stage's last PVs + finish run BEFORE this
                # stage's final ST pair so its oa-psum WAR (DVE copies) clears
                # before the next stage's PV run begins.
                s_last = len(stages) - 1
                for s in range(len(stages)):
                    for jt2 in range(0, NT, 2):
                        issue_st(s, jt2)
                        issue_st(s, jt2 + 1)
                        # fills BEFORE exps: the fill's projection casts then
                        # precede any bitexp in the DVE queue, so the ST pair
                        # that consumes a just-cast qT/kT chunk isn't stuck
                        # behind a 1.2us DVE exp (measured 60ns-slack stalls)
                        for j in (jt2, jt2 + 1):
                            if (s, j) in fill:
                                fill[(s, j)]()
                        issue_exp(s, jt2)
                        issue_exp(s, jt2 + 1)
                        if s > 0 and jt2 in (4, 8):
                            for j in range(jt2 - 4, jt2):
                                issue_pv(s - 1, j)
                        if s > 0 and jt2 == 10:
                            # PVs 8..15 + finish here (not after the jt
                            # loop), c2-SEPARATED: oa[0] completes 8 matmuls
                            # before the run ends, so its CAST+RECIP on DVE
                            # overlap the c2=1 half of the run and the next
                            # stage's first PV (drain included) never
                            # WAR-waits on these oa banks.
                            for c2 in range(2):
                                for j in range(8, NT):
                                    issue_pv_one(s - 1, j, c2)
                            finish_stage(s - 1)
                # last-stage drain, one i-half at a time (16-matmul same-shape
                # runs), with wout tiles INTERLEAVED: OT chunks 0/1 are final
                # for all heads at finish(6) (head 3's half-0 normalize), so
                # wout tiles 0..7 don't depend on the drain at all — running
                # them between the two drain halves starts the 4MB out-DMA
                # stream ~5us earlier, shrinking the post-compute drain.
                h_l, half_l = stages[s_last]
                oa_tiles[s_last] = [
                    oaps.tile([65, 512], f32, tag="oa", name="oa") for _ in range(2)
                ]

                def drain_half(c2):
                    oa = oa_tiles[s_last][c2]
                    for jt in range(NT):
                        p_t, is_u16 = p_tiles[(s_last, jt)]
                        src = p_t[:, c2 * 512:(c2 + 1) * 512]
                        if is_u16:
                            src = src.bitcast(bf16)
                        vbase = jt * (DL + HPC) + h_l * 65
                        nc.tensor.matmul(
                            oa[:],
                            vaug[:, vbase: vbase + 65],
                            src,
                            start=(jt == 0),
                            stop=(jt == NT - 1),
                        )
                    i0 = half_l * 1024 + c2 * 512
                    eng_cp = nc.vector.tensor_copy if c2 == 0 else nc.scalar.copy
                    eng_cp(
                        out=osb2[h_l // 2][(h_l % 2) * D:(h_l % 2) * D + D, i0:i0 + 512],
                        in_=oa[0:D, :],
                    )
                    rec = opool.tile([65, 512], f32, tag="rectmp", name="rec", bufs=3)
                    nc.vector.reciprocal_approx_fast(out=rec[:], in_=oa[:])
                    nc.vector.tensor_copy(
                        out=recstack[h_l * 32:h_l * 32 + 1, i0:i0 + 512],
                        in_=rec[D:D + 1, :],
                    )

                def wout_rows(its, stps_only=False):
                    # output projection, bf16 partials (host sums in fp32).
                    # Post-drain tiles alternate wp between stps and the
                    # then-idle oaps (5 psum slots rotating instead of 3);
                    # tiles interleaved with the drain use stps only (the oa
                    # banks still hold live PV accumulators). Each tile DMAs
                    # as soon as its copies land.
                    for it in its:
                        o_sb = outpool.tile([128, 1024], bf16, tag="o_sb", name="o_sb")
                        use_oa = (it % 2 == 1) and not stps_only
                        for cc in range(2):
                            wp = (oaps if use_oa else stps).tile(
                                [128, 512], f32,
                                tag="oa" if use_oa else "st", name="wp",
                            )
                            for kt in range(2):
                                nc.tensor.matmul(
                                    wp[:],
                                    OT[kt][:, it * 128:(it + 1) * 128],
                                    wo_sb[:, kt * DIM + cc * 512: kt * DIM + (cc + 1) * 512],
                                    start=(kt == 0),
                                    stop=(kt == 1),
                                )
                            # first tiles: both copies on Act — at the
                            # drain->wout transition DVE is still busy with
                            # the drain-half finish chain, and the first
                            # out-DMA (gated on both copies) anchors the
                            # whole 11us+ HBM write stream. Only tiles 0-1:
                            # more makes Act the pacer of the interleaved
                            # wout segment (8 copies vs 4.7us of PE there).
                            if it < 2 or cc == 1:
                                nc.scalar.copy(
                                    out=o_sb[:, cc * 512:(cc + 1) * 512], in_=wp[:]
                                )
                            else:
                                nc.vector.tensor_copy(
                                    out=o_sb[:, cc * 512:(cc + 1) * 512], in_=wp[:]
                                )
                            if it >= 14:
                                # last two tiles: ship each half as soon as
                                # its copy lands — shortens the final DMA
                                # drain after the last compute op
                                nc.sync.dma_start(
                                    out=out[it * 128:(it + 1) * 128,
                                            cc * 512:(cc + 1) * 512],
                                    in_=o_sb[:, cc * 512:(cc + 1) * 512],
                                )
                        if it < 14:
                            nc.sync.dma_start(
                                out=out[it * 128:(it + 1) * 128, :], in_=o_sb[:]
                            )

                drain_half(0)
                wout_rows(range(0, 5), stps_only=True)
                drain_half(1)
                for jt in range(NT):
                    p_tiles.pop((s_last, jt))
                oa_tiles.pop(s_last)
                wout_rows(range(5, 8), stps_only=True)
                normalize(h_l, (2,))
                wout_rows(range(8, 12))
                normalize(h_l, (3,))
                wout_rows(range(12, 16))

    nc.compile()
    return nc


def _get_nc():
    global _cached_nc
    if _cached_nc is None:
        _cached_nc = _build_nc()
    return _cached_nc


def _pack_kt(a):
    """[K, M] -> [128, (K//128)*M] with [p, kt*M + m] = a[kt*128 + p, m]."""
    k, m = a.shape
    return np.ascontiguousarray(
        a.reshape(k // 128, 128, m).transpose(1, 0, 2).reshape(128, -1)
    )


def _make_in_maps(x, W_qkv, W_out):
    bf = ml_dtypes.bfloat16
    in_maps = []
    for c in range(NCORES):
        b, g = c // HPC, c % HPC
        xT = np.ascontiguousarray(x[b].T)  # [DIM, N] fp32
        # [p, q*QW + kt*512 + ii] = xT[kt*128+p, q*512+ii]  (i-quarter-major)
        xtq = xT.reshape(KT, 128, 4, 512).transpose(1, 2, 0, 3).reshape(128, 4 * QW)
        in_maps.append({
            "xt": np.ascontiguousarray(xtq).astype(bf),
            "wq": _pack_kt(W_qkv[:, g * DL:(g + 1) * DL] * SCALE).astype(bf),
            "wk": _pack_kt(W_qkv[:, DIM + g * DL: DIM + (g + 1) * DL]).astype(bf),
            "wv": _pack_kt(W_qkv[:, 2 * DIM + g * DL: 2 * DIM + (g + 1) * DL]).astype(bf),
            "wo": _pack_kt(W_out[g * DL:(g + 1) * DL, :]).astype(bf),
            "onesmask": _ones_mask(),
        })
    return in_maps


def _ones_mask():
    """[128, HPC*128]: block h has ones in row 32h, cols 0..D-1."""
    m = np.zeros((128, HPC * 128), np.float32)
    for h in range(HPC):
        m[h * 32, h * 128:h * 128 + D] = 1.0
    return m.astype(ml_dtypes.bfloat16)


def _run(x, W_qkv, W_out, b_out, trace=False):
    from concourse.bass_utils import run_bass_kernel_spmd

    nc = _get_nc()
    in_maps = _make_in_maps(x, W_qkv, W_out)
    res = run_bass_kernel_spmd(nc, in_maps, core_ids=list(range(NCORES)), trace=trace)
    y = np.zeros((B, N, DIM), np.float32)
    for c in range(NCORES):
        y[c // HPC] += res.results[c]["out"].astype(np.float32)
    y += b_out.astype(np.float32)[None, None, :]
    return y, res


def _numpy_reference(x, mask, W_qkv, W_out, b_out):
    """Slow exact fallback (only used if mask is not all-True)."""
    b, n, dim = x.shape
    d = dim // H
    qkv = x @ W_qkv
    q, k, v = np.split(qkv, 3, axis=-1)
    th = lambda t: t.reshape(b, n, H, d).transpose(0, 2, 1, 3)
    q, k, v = th(q), th(k), th(v)
    dots = np.einsum('bhid,bhjd->bhij', q, k) * (d ** -0.5)
    dots = np.where(mask[:, None, None, :], dots, -np.finfo(np.float32).max)
    dots -= dots.max(-1, keepdims=True)
    e = np.exp(dots)
    attn = e / e.sum(-1, keepdims=True)
    o = np.einsum('bhij,bhjd->bhid', attn, v)
    o = o.transpose(0, 2, 1, 3).reshape(b, n, dim)
    return o @ W_out + b_out


def kernel(x, mask, W_qkv, W_out, b_out):
    x = np.asarray(x, np.float32)
    mask = np.asarray(mask)
    W_qkv = np.asarray(W_qkv, np.float32)
    W_out = np.asarray(W_out, np.float32)
    b_out = np.asarray(b_out, np.float32)
    assert x.shape == (B, N, DIM) and W_qkv.shape == (DIM, 3 * DIM)
    if not mask.all():
        return _numpy_reference(x, mask, W_qkv, W_out, b_out).astype(np.float32)
    y, _ = _run(x, W_qkv, W_out, b_out, trace=False)
    return y

